# revision 20
# baseline (speedup 1.0000x reference)
"""MoE (dense-act-dense, top-4 of 8 experts) Trainium2 kernel.

Strategy (expert-parallel, host-side dispatch, load-balanced):
  - The forward combine weight is exactly 1.0 (straight-through gate trick in
    the reference), so out[n] = sum_{e in top4(n)} expert_e(x[n]).
  - Host computes the tiny gate matmul + top-4 routing (0.05% of FLOPs) and
    dispatches tokens: core e receives the tokens routed to expert e, plus
    expert e's weights. Each of the 8 cores runs a dense 2-layer MLP:
      h = relu(w1[e] @ x) ; y = w2[e] @ h
    as two chained bf16 GEMMs (bf16 data, fp32 PSUM accumulate). bf16 halves
    DMA traffic + SBUF vs fp32r at the same 1 cycle/row PE rate, and its
    ~3e-3 rel-err is far inside the 2e-2 gate.
  - Load balancing: instead of padding every core to the max expert load,
    each core gets PRIM primary tokens (its own expert) plus one V-wide
    overflow tile holding surplus tokens of ONE (possibly different) expert,
    with that expert's weights as a second input set. (PRIM, V) minimizes
    PRIM+V subject to the surpluses packing into 8 single-expert bins.
  - Host scatter-adds per-expert outputs back (weight 1.0 per selection).

Per-core device layouts (host pre-arranges everything for contiguous DMA):
  xR{t} [P, DC, w_t] bf16 : tile t's tokens, pre-swizzled so each partition
                            is one contiguous multi-KB DMA run (descriptor-
                            dispatch rate limits the startup otherwise).
  w1r/w1o [H, D] bf16 : slice-major stationary layout; rows hc*128+k hold
                        w1[e][hc*128+m, dc*128+k] at col dc*128+m, so a
                        128-col PE slice DMAs as one 4KB run per partition.
  w2r/w2o [O, H] bf16 : same trick for layer 2 (2KB runs).
  yT  [O, C] f32 : expert output, transposed.

Schedule notes:
  - Tile widths <= 512 (PSUM bank limit): measured per-mm spacing is
    ~(NT+10)cyc in bf16, so the widest legal tiles minimize the bubble.
  - ~10 dummy matmuls on a memset scratch tile fill the ~12us DMA startup
    window so the PE p-state is fully ramped (2.4GHz) when real work starts;
    a cold or idle-reset PE runs ~2x slow for its first ~3us.
  - ONE input fifo (sync queue) ordered by need-time: w1 slice 0 is split in
    dc quarters ahead of x0's first chunk so the first gemm chain starts
    after ~0.6MB; splitting across rings splits HBM bandwidth and starves
    the critical prefix (measured).
  - GEMM2(t) is emitted one tile behind GEMM1(t+1) (depth-1 software
    pipeline) to give the PE GEMM1 work while w2 is still streaming in.
  - y drains: PSUM -> SBUF copy on vector, store DMA issued on scalar, so
    the sync queue (x + weights, latency-critical) never blocks behind them.
"""

import numpy as np
import ml_dtypes
from contextlib import ExitStack

import concourse.bass as bass
import concourse.tile as tile
from concourse import bacc, mybir
from concourse import bass_utils

F32 = mybir.dt.float32
BF16 = mybir.dt.bfloat16
P = 128

TOP_K = 4
D, H, O, E = 2048, 1024, 2048, 8
_NC_CACHE = {}
NPBF16 = ml_dtypes.bfloat16


def _tile_widths(C, target=512):
    """Split C tokens (padded to even) into even tiles of near-equal width
    <= target (PSUM bank holds 512 fp32)."""
    C = max(C + (C % 2), 256)
    C2 = C // 2
    ntiles = -(-C // target)
    base = C2 // ntiles
    rem = C2 - base * ntiles
    widths = [2 * (base + 1)] * rem + [2 * base] * (ntiles - rem)
    widths.sort(reverse=True)
    assert sum(widths) == C and all(w <= target and w % 2 == 0 for w in widths)
    return widths


def _plan(counts):
    """Pick (PRIM, V): every core does PRIM primary + V overflow tokens.
    Surpluses above PRIM must pack into <= 8 single-expert bins of size V.
    Returns (PRIM, V, chunks) with chunks = [(expert, off, take), ...]."""
    best = None
    lo = (int(min(counts.mean(), counts.min() + 512)) - 64) & ~1
    for PRIM in range(max(256, lo), int(counts.max()) + 2, 2):
        s = [max(0, int(c) - PRIM) for c in counts]
        S = sum(s)
        if S == 0:
            V = 0
        else:
            V = max(2, 2 * (-(-S // (2 * len(counts)))))
            while sum(-(-si // V) for si in s if si) > len(counts):
                V += 2
        if best is None or PRIM + V < best[0] + best[1]:
            best = (PRIM, V)
    PRIM, V = best
    chunks = []
    for e, c in enumerate(counts):
        se, off = max(0, int(c) - PRIM), 0
        while se > 0:
            take = min(se, V)
            chunks.append((e, off, take))
            off += take
            se -= take
    assert len(chunks) <= len(counts)
    return PRIM, V, chunks


def build_expert_kernel(PRIM, V, target=512):
    """Per-core program: dense [C, D] @ [D, H] -> relu -> @ [H, O] in bf16,
    where C = PRIM tokens with the primary weights + V with the overflow
    weights (V may be 0)."""
    DC, HC, OC = D // P, H // P, O // P
    XG = 4  # dc-groups per x tile (chunked DMA for startup pipelining)
    widths = _tile_widths(PRIM, target) + ([V] if V else [])
    starts = [sum(widths[:i]) for i in range(len(widths))]
    NTILES = len(widths)
    NP = NTILES - (1 if V else 0)  # tiles using the primary weights
    NTMAX = max(widths)
    nc = bacc.Bacc("TRN2", target_bir_lowering=False, debug=False, num_devices=E)
    xR = [
        nc.dram_tensor(f"xR{t}", [P, DC, w], BF16, kind="ExternalInput").ap()
        for t, w in enumerate(widths)
    ]
    w1r = nc.dram_tensor("w1r", [H, D], BF16, kind="ExternalInput").ap()
    w2r = nc.dram_tensor("w2r", [O, H], BF16, kind="ExternalInput").ap()
    if V:
        w1o = nc.dram_tensor("w1o", [H, D], BF16, kind="ExternalInput").ap()
        w2o = nc.dram_tensor("w2o", [O, H], BF16, kind="ExternalInput").ap()
    yT = nc.dram_tensor("yT", [O, PRIM + V], F32, kind="ExternalOutput").ap()

    with tile.TileContext(nc) as tc, ExitStack() as ctx:
        dpool = ctx.enter_context(tc.tile_pool(name="d", bufs=1))
        wpool = ctx.enter_context(tc.tile_pool(name="w", bufs=1))
        xpool = ctx.enter_context(tc.tile_pool(name="x", bufs=3 * XG))
        hpool = ctx.enter_context(tc.tile_pool(name="h", bufs=2))
        ypool = ctx.enter_context(tc.tile_pool(name="y", bufs=4))
        psd = ctx.enter_context(tc.tile_pool(name="psd", bufs=1, space="PSUM"))
        ps1 = ctx.enter_context(tc.tile_pool(name="ps1", bufs=2, space="PSUM"))
        ps2 = ctx.enter_context(tc.tile_pool(name="ps2", bufs=4, space="PSUM"))

        # --- PE p-state warmup (see module docstring) ---
        dum = dpool.tile([P, 512], BF16, name="dum")
        nc.gpsimd.memset(dum[:], 0.0)
        pd = psd.tile([P, 512], F32, name="pd")
        NWARM = 10
        for i in range(NWARM):
            nc.tensor.matmul(
                pd[:], dum[:, 0:P], dum[:],
                start=(i == 0), stop=(i == NWARM - 1),
            )

        x_tiles = {}

        def dma_x(t):
            w_t = widths[t]
            G = DC // XG
            chunks = []
            for g in range(XG):
                xc = xpool.tile([P, G, NTMAX], BF16, name="x_t")[:, :, :w_t]
                nc.sync.dma_start(xc[:], xR[t][:, g * G:(g + 1) * G, :])
                chunks.append(xc)
            x_tiles[t] = chunks

        def dma_w1(src, hc, out, split=1):
            w = wpool.tile([P, DC, P], BF16,
                           name=f"w1{'o' if out is w1os else 's'}{hc}")
            for i in range(split):
                dc0, dc1 = i * DC // split, (i + 1) * DC // split
                nc.sync.dma_start(
                    w[:, dc0:dc1, :],
                    src[hc * P:(hc + 1) * P,
                        dc0 * P:dc1 * P].rearrange("p (dc j) -> p dc j", j=P),
                )
            out[hc] = w

        def dma_w2(src, oc, out):
            w = wpool.tile([P, HC, P], BF16,
                           name=f"w2{'o' if out is w2os else 's'}{oc}")
            nc.sync.dma_start(
                w[:],
                src[oc * P:(oc + 1) * P, :].rearrange("p (hc j) -> p hc j", hc=HC),
            )
            out[oc] = w

        # --- startup DMA stream: ONE fifo, ordered by need-time. w1 slice 0
        # is dc-split so the first chain's first matmuls wait on ~0.6MB. ---
        w1s, w2s = [None] * HC, [None] * OC
        w1os, w2os = [None] * HC, [None] * OC
        dma_w1(w1r, 0, w1s, split=4)
        dma_x(0)
        for hc in range(1, HC):
            dma_w1(w1r, hc, w1s)
        if NTILES > 1:
            dma_x(1)
        for oc in range(OC):
            dma_w2(w2r, oc, w2s)
        if V:
            for hc in range(HC):
                dma_w1(w1o, hc, w1os)
            for oc in range(OC):
                dma_w2(w2o, oc, w2os)

        def gemm1(t):
            w_t = widths[t]
            xc = x_tiles.pop(t)
            ws = w1os if t >= NP else w1s
            h_t = hpool.tile([P, HC, NTMAX], BF16, name="h_t")[:, :, :w_t]
            for hc in range(HC):
                ph = ps1.tile([P, NTMAX], F32, name="ph")[:, :w_t]
                for dc in range(DC):
                    nc.tensor.matmul(
                        ph[:], ws[hc][:, dc, :], xc[dc // XG][:, dc % XG, :],
                        start=(dc == 0), stop=(dc == DC - 1),
                    )
                nc.scalar.activation(
                    h_t[:, hc, :], ph[:], mybir.ActivationFunctionType.Relu
                )
            return h_t

        def gemm2(t, h_t, last=False):
            w_t = widths[t]
            ws = w2os if t >= NP else w2s
            for oc in range(OC):
                # split the very last chain in half so the post-PE drain
                # (PSUM copy + store) runs on a half-width tile
                splits = [0, w_t // 2 - (w_t // 2) % 2, w_t] \
                    if (last and oc == OC - 1 and w_t > 64) else [0, w_t]
                for a, b in zip(splits, splits[1:]):
                    po = ps2.tile([P, NTMAX], F32, name="po")[:, :b - a]
                    for hc in range(HC):
                        nc.tensor.matmul(
                            po[:], ws[oc][:, hc, :], h_t[:, hc, a:b],
                            start=(hc == 0), stop=(hc == HC - 1),
                        )
                    y_t = ypool.tile([P, NTMAX], F32, name="y_t")[:, :b - a]
                    nc.vector.tensor_copy(y_t[:], po[:])
                    nc.scalar.dma_start(
                        yT[oc * P:(oc + 1) * P,
                           starts[t] + a:starts[t] + b], y_t[:]
                    )

        # --- depth-1 software-pipelined main loop ---
        h_tiles = {}
        for t in range(NTILES):
            h_tiles[t] = gemm1(t)
            if t >= 1:
                gemm2(t - 1, h_tiles.pop(t - 1))
            if t + 2 < NTILES:
                dma_x(t + 2)
        gemm2(NTILES - 1, h_tiles.pop(NTILES - 1), last=True)
    nc.compile()
    return nc


def _route(xt, wg):
    """Host-side gate + top-4. Gap between 4th/5th gate values is ~3e-5 for
    this distribution, far above fp32 matmul noise, so fp32 reproduces the
    reference top-k set exactly."""
    gate = xt @ wg  # [N, E] fp32
    top4 = np.argpartition(-gate, TOP_K - 1, axis=1)[:, :TOP_K]  # set, unordered
    return top4


def _w1_slice_major(w1e):
    """[H, D] -> rows hc*128+k, cols dc*128+m = w1e[hc*128+m, dc*128+k]."""
    HC, DC = H // P, D // P
    return np.ascontiguousarray(
        w1e.reshape(HC, P, DC, P).transpose(0, 3, 2, 1).reshape(H, D)
    )


def _w2_slice_major(w2e):
    """[O, H] -> rows oc*128+k, cols hc*128+m = w2e[oc*128+m, hc*128+k]."""
    OC, HC = O // P, H // P
    return np.ascontiguousarray(
        w2e.reshape(OC, P, HC, P).transpose(0, 3, 2, 1).reshape(O, H)
    )


def _x_tiles(xe, widths):
    """[C, D] tokens -> per-tile [P, DC, w] with per-partition contiguity."""
    out = {}
    Dx = xe.shape[1]
    s0 = 0
    for t, w in enumerate(widths):
        out[f"xR{t}"] = np.ascontiguousarray(
            xe[s0:s0 + w].T.reshape(Dx // P, P, w).transpose(1, 0, 2)
        )
        s0 += w
    return out


def kernel(x, wg, w1, w2, _want_results=False, _run_kwargs=None):
    x = np.asarray(x, dtype=np.float32)
    wg = np.asarray(wg, dtype=np.float32)
    w1 = np.asarray(w1, dtype=np.float32)
    w2 = np.asarray(w2, dtype=np.float32)
    B, S, Dx = x.shape
    N = B * S
    xt = np.ascontiguousarray(x.reshape(N, Dx))
    top4 = _route(xt, wg)

    # token lists per expert
    sel = np.zeros((N, E), dtype=bool)
    np.put_along_axis(sel, top4, True, axis=1)
    tokens = [np.nonzero(sel[:, e])[0] for e in range(E)]
    counts = np.array([len(t) for t in tokens])

    PRIM, V, chunks = _plan(counts)
    if (PRIM, V) not in _NC_CACHE:
        try:
            _NC_CACHE[(PRIM, V)] = build_expert_kernel(PRIM, V)
        except ValueError:
            # SBUF pressure fallback: no balancing, pad to the max load
            CAP = max(int(counts.max()), 256)
            CAP += CAP % 2
            PRIM, V, chunks = CAP, 0, []
            if (PRIM, V) not in _NC_CACHE:
                _NC_CACHE[(PRIM, V)] = build_expert_kernel(PRIM, V)
    nc = _NC_CACHE[(PRIM, V)]
    widths = _tile_widths(PRIM) + ([V] if V else [])

    xbf = xt.astype(NPBF16)
    w1bf = [_w1_slice_major(w1[e].astype(NPBF16)) for e in range(E)]
    w2bf = [_w2_slice_major(w2[e].astype(NPBF16)) for e in range(E)]
    in_maps = []
    core_chunks = []
    for c in range(E):
        ch = chunks[c] if c < len(chunks) else (c, 0, 0)
        core_chunks.append(ch)
        e2, off, take = ch
        xe = np.zeros((PRIM + V, Dx), dtype=NPBF16)
        nprim = min(int(counts[c]), PRIM)
        xe[:nprim] = xbf[tokens[c][:nprim]]
        if take:
            xe[PRIM:PRIM + take] = xbf[tokens[e2][PRIM + off:PRIM + off + take]]
        im = {"w1r": w1bf[c], "w2r": w2bf[c]}
        if V:
            im["w1o"] = w1bf[e2]
            im["w2o"] = w2bf[e2]
        im.update(_x_tiles(xe, widths))
        in_maps.append(im)

    res = bass_utils.run_bass_kernel_spmd(
        nc, in_maps, core_ids=list(range(E)), **(_run_kwargs or {})
    )

    out = np.zeros((N, O), dtype=np.float32)
    for c in range(E):
        yTc = res.results[c]["yT"].T
        nprim = min(int(counts[c]), PRIM)
        out[tokens[c][:nprim]] += yTc[:nprim]
        e2, off, take = core_chunks[c]
        if take:
            out[tokens[e2][PRIM + off:PRIM + off + take]] += \
                yTc[PRIM:PRIM + take]
    out = out.reshape(B, S, O)
    if _want_results:
        return out, res
    return out


# revision 24
# speedup vs baseline: 1.0193x; 1.0193x over previous
"""MoE (dense-act-dense, top-4 of 8 experts) Trainium2 kernel.

Strategy (expert-parallel, host-side dispatch, load-balanced):
  - The forward combine weight is exactly 1.0 (straight-through gate trick in
    the reference), so out[n] = sum_{e in top4(n)} expert_e(x[n]).
  - Host computes the tiny gate matmul + top-4 routing (0.05% of FLOPs) and
    dispatches tokens: core e receives the tokens routed to expert e, plus
    expert e's weights. Each of the 8 cores runs a dense 2-layer MLP:
      h = relu(w1[e] @ x) ; y = w2[e] @ h
    as two chained bf16 GEMMs (bf16 data, fp32 PSUM accumulate). bf16 halves
    DMA traffic + SBUF vs fp32r at the same 1 cycle/row PE rate, and its
    ~3e-3 rel-err is far inside the 2e-2 gate.
  - Load balancing: instead of padding every core to the max expert load,
    each core gets PRIM primary tokens (its own expert) plus one V-wide
    overflow tile holding surplus tokens of ONE (possibly different) expert,
    with that expert's weights as a second input set. (PRIM, V) minimizes
    PRIM+V subject to the surpluses packing into 8 single-expert bins.
  - Host scatter-adds per-expert outputs back (weight 1.0 per selection).

Per-core device layouts (host pre-arranges everything for contiguous DMA):
  xR{t} [P, DC, w_t] bf16 : tile t's tokens, pre-swizzled so each partition
                            is one contiguous multi-KB DMA run (descriptor-
                            dispatch rate limits the startup otherwise).
  w1r/w1o [H, D] bf16 : slice-major stationary layout; rows hc*128+k hold
                        w1[e][hc*128+m, dc*128+k] at col dc*128+m, so a
                        128-col PE slice DMAs as one 4KB run per partition.
  w2r/w2o [O, H] bf16 : same trick for layer 2 (2KB runs).
  yT  [O, C] f32 : expert output, transposed.

Schedule notes:
  - Tile widths <= 512 (PSUM bank limit): measured per-mm spacing is
    ~(NT+10)cyc in bf16, so the widest legal tiles minimize the bubble.
  - ~10 dummy matmuls on a memset scratch tile fill the ~12us DMA startup
    window so the PE p-state is fully ramped (2.4GHz) when real work starts;
    a cold or idle-reset PE runs ~2x slow for its first ~3us.
  - ONE input fifo (sync queue) ordered by need-time: w1 slice 0 is split in
    dc quarters ahead of x0's first chunk so the first gemm chain starts
    after ~0.6MB; splitting across rings splits HBM bandwidth and starves
    the critical prefix (measured).
  - GEMM2(t) is emitted one tile behind GEMM1(t+1) (depth-1 software
    pipeline) to give the PE GEMM1 work while w2 is still streaming in.
  - y drains: PSUM -> SBUF copy on vector, store DMA issued on scalar, so
    the sync queue (x + weights, latency-critical) never blocks behind them.
"""

import numpy as np
import ml_dtypes
from contextlib import ExitStack

import concourse.bass as bass
import concourse.tile as tile
from concourse import bacc, mybir
from concourse import bass_utils

F32 = mybir.dt.float32
BF16 = mybir.dt.bfloat16
P = 128

TOP_K = 4
D, H, O, E = 2048, 1024, 2048, 8
_NC_CACHE = {}
NPBF16 = ml_dtypes.bfloat16
_BALANCE = False


def _tile_widths(C, target=512):
    """Split C tokens (padded to even) into even tiles of near-equal width
    <= target (PSUM bank holds 512 fp32)."""
    C = max(C + (C % 2), 256)
    C2 = C // 2
    ntiles = -(-C // target)
    base = C2 // ntiles
    rem = C2 - base * ntiles
    widths = [2 * (base + 1)] * rem + [2 * base] * (ntiles - rem)
    widths.sort(reverse=True)
    assert sum(widths) == C and all(w <= target and w % 2 == 0 for w in widths)
    return widths


def _plan(counts):
    """Pick (PRIM, V): every core does PRIM primary + V overflow tokens.
    Surpluses above PRIM must pack into <= 8 single-expert bins of size V.
    Returns (PRIM, V, chunks) with chunks = [(expert, off, take), ...]."""
    best = None
    lo = (int(min(counts.mean(), counts.min() + 512)) - 64) & ~1
    for PRIM in range(max(256, lo), int(counts.max()) + 2, 2):
        s = [max(0, int(c) - PRIM) for c in counts]
        S = sum(s)
        if S == 0:
            V = 0
        else:
            V = max(2, 2 * (-(-S // (2 * len(counts)))))
            while sum(-(-si // V) for si in s if si) > len(counts):
                V += 2
        if best is None or PRIM + V < best[0] + best[1]:
            best = (PRIM, V)
    PRIM, V = best
    chunks = []
    for e, c in enumerate(counts):
        se, off = max(0, int(c) - PRIM), 0
        while se > 0:
            take = min(se, V)
            chunks.append((e, off, take))
            off += take
            se -= take
    assert len(chunks) <= len(counts)
    return PRIM, V, chunks


def build_expert_kernel(PRIM, V, target=512):
    """Per-core program: dense [C, D] @ [D, H] -> relu -> @ [H, O] in bf16,
    where C = PRIM tokens with the primary weights + V with the overflow
    weights (V may be 0)."""
    DC, HC, OC = D // P, H // P, O // P
    XG = 4  # dc-groups per x tile (chunked DMA for startup pipelining)
    widths = _tile_widths(PRIM, target) + ([V] if V else [])
    starts = [sum(widths[:i]) for i in range(len(widths))]
    NTILES = len(widths)
    NP = NTILES - (1 if V else 0)  # tiles using the primary weights
    NTMAX = max(widths)
    nc = bacc.Bacc("TRN2", target_bir_lowering=False, debug=False, num_devices=E)
    xR = [
        nc.dram_tensor(f"xR{t}", [P, DC, w], BF16, kind="ExternalInput").ap()
        for t, w in enumerate(widths)
    ]
    w1r = nc.dram_tensor("w1r", [H, D], BF16, kind="ExternalInput").ap()
    w2r = nc.dram_tensor("w2r", [O, H], BF16, kind="ExternalInput").ap()
    if V:
        w1o = nc.dram_tensor("w1o", [H, D], BF16, kind="ExternalInput").ap()
        w2o = nc.dram_tensor("w2o", [O, H], BF16, kind="ExternalInput").ap()
    yT = nc.dram_tensor("yT", [O, PRIM + V], F32, kind="ExternalOutput").ap()

    with tile.TileContext(nc) as tc, ExitStack() as ctx:
        dpool = ctx.enter_context(tc.tile_pool(name="d", bufs=1))
        wpool = ctx.enter_context(tc.tile_pool(name="w", bufs=1))
        xpool = ctx.enter_context(tc.tile_pool(name="x", bufs=3 * XG))
        hpool = ctx.enter_context(tc.tile_pool(name="h", bufs=2))
        ypool = ctx.enter_context(tc.tile_pool(name="y", bufs=4))
        psd = ctx.enter_context(tc.tile_pool(name="psd", bufs=1, space="PSUM"))
        ps1 = ctx.enter_context(tc.tile_pool(name="ps1", bufs=2, space="PSUM"))
        ps2 = ctx.enter_context(tc.tile_pool(name="ps2", bufs=4, space="PSUM"))

        # --- PE p-state warmup (see module docstring) ---
        dum = dpool.tile([P, 512], BF16, name="dum")
        nc.gpsimd.memset(dum[:], 0.0)
        pd = psd.tile([P, 512], F32, name="pd")
        NWARM = 14
        for i in range(NWARM):
            nc.tensor.matmul(
                pd[:], dum[:, 0:P], dum[:],
                start=(i == 0), stop=(i == NWARM - 1),
            )

        x_tiles = {}

        def dma_x(t):
            w_t = widths[t]
            G = DC // XG
            chunks = []
            for g in range(XG):
                xc = xpool.tile([P, G, NTMAX], BF16, name="x_t")[:, :, :w_t]
                nc.sync.dma_start(xc[:], xR[t][:, g * G:(g + 1) * G, :])
                chunks.append(xc)
            x_tiles[t] = chunks

        def dma_w1(src, hc, out, split=1):
            w = wpool.tile([P, DC, P], BF16,
                           name=f"w1{'o' if out is w1os else 's'}{hc}")
            for i in range(split):
                dc0, dc1 = i * DC // split, (i + 1) * DC // split
                nc.sync.dma_start(
                    w[:, dc0:dc1, :],
                    src[hc * P:(hc + 1) * P,
                        dc0 * P:dc1 * P].rearrange("p (dc j) -> p dc j", j=P),
                )
            out[hc] = w

        def dma_w2(src, oc, out):
            w = wpool.tile([P, HC, P], BF16,
                           name=f"w2{'o' if out is w2os else 's'}{oc}")
            nc.sync.dma_start(
                w[:],
                src[oc * P:(oc + 1) * P, :].rearrange("p (hc j) -> p hc j", hc=HC),
            )
            out[oc] = w

        # --- startup DMA stream: ONE fifo, ordered by need-time. w1 slice 0
        # is dc-split so the first chain's first matmuls wait on ~0.6MB. ---
        w1s, w2s = [None] * HC, [None] * OC
        w1os, w2os = [None] * HC, [None] * OC
        dma_w1(w1r, 0, w1s)
        dma_x(0)
        for hc in range(1, HC):
            dma_w1(w1r, hc, w1s)
        if NTILES > 1:
            dma_x(1)
        for oc in range(OC):
            dma_w2(w2r, oc, w2s)
        if V:
            for hc in range(HC):
                dma_w1(w1o, hc, w1os)
            for oc in range(OC):
                dma_w2(w2o, oc, w2os)

        def gemm1(t):
            w_t = widths[t]
            xc = x_tiles.pop(t)
            ws = w1os if t >= NP else w1s
            h_t = hpool.tile([P, HC, NTMAX], BF16, name="h_t")[:, :, :w_t]
            for hc in range(HC):
                ph = ps1.tile([P, NTMAX], F32, name="ph")[:, :w_t]
                for dc in range(DC):
                    nc.tensor.matmul(
                        ph[:], ws[hc][:, dc, :], xc[dc // XG][:, dc % XG, :],
                        start=(dc == 0), stop=(dc == DC - 1),
                    )
                nc.scalar.activation(
                    h_t[:, hc, :], ph[:], mybir.ActivationFunctionType.Relu
                )
            return h_t

        def gemm2(t, h_t, last=False):
            w_t = widths[t]
            ws = w2os if t >= NP else w2s
            for oc in range(OC):
                # split the very last chain in half so the post-PE drain
                # (PSUM copy + store) runs on a half-width tile
                splits = [0, w_t // 2 - (w_t // 2) % 2, w_t] \
                    if (last and oc == OC - 1 and w_t > 64) else [0, w_t]
                for a, b in zip(splits, splits[1:]):
                    po = ps2.tile([P, NTMAX], F32, name="po")[:, :b - a]
                    for hc in range(HC):
                        nc.tensor.matmul(
                            po[:], ws[oc][:, hc, :], h_t[:, hc, a:b],
                            start=(hc == 0), stop=(hc == HC - 1),
                        )
                    y_t = ypool.tile([P, NTMAX], F32, name="y_t")[:, :b - a]
                    nc.vector.tensor_copy(y_t[:], po[:])
                    nc.scalar.dma_start(
                        yT[oc * P:(oc + 1) * P,
                           starts[t] + a:starts[t] + b], y_t[:]
                    )

        # --- depth-1 software-pipelined main loop ---
        h_tiles = {}
        for t in range(NTILES):
            h_tiles[t] = gemm1(t)
            if t >= 1:
                gemm2(t - 1, h_tiles.pop(t - 1))
            if t + 2 < NTILES:
                dma_x(t + 2)
        gemm2(NTILES - 1, h_tiles.pop(NTILES - 1), last=True)
    nc.compile()
    return nc


def _route(xt, wg):
    """Host-side gate + top-4. Gap between 4th/5th gate values is ~3e-5 for
    this distribution, far above fp32 matmul noise, so fp32 reproduces the
    reference top-k set exactly."""
    gate = xt @ wg  # [N, E] fp32
    top4 = np.argpartition(-gate, TOP_K - 1, axis=1)[:, :TOP_K]  # set, unordered
    return top4


def _w1_slice_major(w1e):
    """[H, D] -> rows hc*128+k, cols dc*128+m = w1e[hc*128+m, dc*128+k]."""
    HC, DC = H // P, D // P
    return np.ascontiguousarray(
        w1e.reshape(HC, P, DC, P).transpose(0, 3, 2, 1).reshape(H, D)
    )


def _w2_slice_major(w2e):
    """[O, H] -> rows oc*128+k, cols hc*128+m = w2e[oc*128+m, hc*128+k]."""
    OC, HC = O // P, H // P
    return np.ascontiguousarray(
        w2e.reshape(OC, P, HC, P).transpose(0, 3, 2, 1).reshape(O, H)
    )


def _x_tiles(xe, widths):
    """[C, D] tokens -> per-tile [P, DC, w] with per-partition contiguity."""
    out = {}
    Dx = xe.shape[1]
    s0 = 0
    for t, w in enumerate(widths):
        out[f"xR{t}"] = np.ascontiguousarray(
            xe[s0:s0 + w].T.reshape(Dx // P, P, w).transpose(1, 0, 2)
        )
        s0 += w
    return out


def kernel(x, wg, w1, w2, _want_results=False, _run_kwargs=None):
    x = np.asarray(x, dtype=np.float32)
    wg = np.asarray(wg, dtype=np.float32)
    w1 = np.asarray(w1, dtype=np.float32)
    w2 = np.asarray(w2, dtype=np.float32)
    B, S, Dx = x.shape
    N = B * S
    xt = np.ascontiguousarray(x.reshape(N, Dx))
    top4 = _route(xt, wg)

    # token lists per expert
    sel = np.zeros((N, E), dtype=bool)
    np.put_along_axis(sel, top4, True, axis=1)
    tokens = [np.nonzero(sel[:, e])[0] for e in range(E)]
    counts = np.array([len(t) for t in tokens])

    if _BALANCE:
        PRIM, V, chunks = _plan(counts)
    else:
        CAP = max(int(counts.max()), 256)
        PRIM, V, chunks = CAP + CAP % 2, 0, []
    if (PRIM, V) not in _NC_CACHE:
        try:
            _NC_CACHE[(PRIM, V)] = build_expert_kernel(PRIM, V)
        except ValueError:
            # SBUF pressure fallback: no balancing, pad to the max load
            CAP = max(int(counts.max()), 256)
            CAP += CAP % 2
            PRIM, V, chunks = CAP, 0, []
            if (PRIM, V) not in _NC_CACHE:
                _NC_CACHE[(PRIM, V)] = build_expert_kernel(PRIM, V)
    nc = _NC_CACHE[(PRIM, V)]
    widths = _tile_widths(PRIM) + ([V] if V else [])

    xbf = xt.astype(NPBF16)
    w1bf = [_w1_slice_major(w1[e].astype(NPBF16)) for e in range(E)]
    w2bf = [_w2_slice_major(w2[e].astype(NPBF16)) for e in range(E)]
    in_maps = []
    core_chunks = []
    for c in range(E):
        ch = chunks[c] if c < len(chunks) else (c, 0, 0)
        core_chunks.append(ch)
        e2, off, take = ch
        xe = np.zeros((PRIM + V, Dx), dtype=NPBF16)
        nprim = min(int(counts[c]), PRIM)
        xe[:nprim] = xbf[tokens[c][:nprim]]
        if take:
            xe[PRIM:PRIM + take] = xbf[tokens[e2][PRIM + off:PRIM + off + take]]
        im = {"w1r": w1bf[c], "w2r": w2bf[c]}
        if V:
            im["w1o"] = w1bf[e2]
            im["w2o"] = w2bf[e2]
        im.update(_x_tiles(xe, widths))
        in_maps.append(im)

    res = bass_utils.run_bass_kernel_spmd(
        nc, in_maps, core_ids=list(range(E)), **(_run_kwargs or {})
    )

    out = np.zeros((N, O), dtype=np.float32)
    for c in range(E):
        yTc = res.results[c]["yT"].T
        nprim = min(int(counts[c]), PRIM)
        out[tokens[c][:nprim]] += yTc[:nprim]
        e2, off, take = core_chunks[c]
        if take:
            out[tokens[e2][PRIM + off:PRIM + off + take]] += \
                yTc[PRIM:PRIM + take]
    out = out.reshape(B, S, O)
    if _want_results:
        return out, res
    return out


# revision 78
# speedup vs baseline: 1.0752x; 1.0548x over previous
"""MoE (dense-act-dense, top-4 of 8 experts) Trainium2 kernel.

Strategy (expert-parallel, host-side dispatch, load-balanced):
  - The forward combine weight is exactly 1.0 (straight-through gate trick in
    the reference), so out[n] = sum_{e in top4(n)} expert_e(x[n]).
  - Host computes the tiny gate matmul + top-4 routing (0.05% of FLOPs) and
    dispatches tokens: core e receives the tokens routed to expert e, plus
    expert e's weights. Each of the 8 cores runs a dense 2-layer MLP:
      h = relu(w1[e] @ x) ; y = w2[e] @ h
    as two chained GEMMs (fp32 PSUM accumulate): bf16 halves DMA + SBUF vs
    fp32r at the same 1 cycle/row PE rate, and 1/4 of gemm1's contraction
    runs in fp8e4m3 DoubleRow at 2x rate (see FP8_DC below).
  - Every core is padded to the max expert load (~1.2% imbalance). A
    load-balanced variant (_BALANCE: overflow columns with a second expert's
    weights) measured SLOWER — see the note at _BALANCE below.
  - Host scatter-adds per-expert outputs back (weight 1.0 per selection).

Per-core device layouts (host pre-arranges everything for contiguous DMA):
  xR{t} [P, DC, w_t] bf16 : tile t's tokens, pre-swizzled so each partition
                            is one contiguous multi-KB DMA run (descriptor-
                            dispatch rate limits the startup otherwise).
  w1r/w1o [H, D] bf16 : slice-major stationary layout; rows hc*128+k hold
                        w1[e][hc*128+m, dc*128+k] at col dc*128+m, so a
                        128-col PE slice DMAs as one 4KB run per partition.
  w2r/w2o [O, H] bf16 : same trick for layer 2 (2KB runs).
  yT  [O, C] f32 : expert output, transposed.

Schedule notes:
  - Tile widths <= 512 (PSUM bank limit): measured per-mm spacing is
    ~(NT+10)cyc in bf16, so the widest legal tiles minimize the bubble.
  - ~13 dummy matmuls on a memset scratch tile fill the startup window so
    the PE p-state is fully ramped (2.4GHz) when real work starts; the PE
    runs ~2x slow until ~6us of continuous busy time, and that ramp — not
    DMA — is the startup floor once the engines are primed (a gpsimd-queue
    priming DMA absorbs the ~0.8us per-engine cold-start in parallel with
    the sync ring opening). Starting real work before the ramp completes
    just pays the slow cycles on real matmuls (measured).
  - ONE input fifo (sync queue) ordered by need-time; splitting across
    rings splits HBM bandwidth and starves the critical prefix (measured).
  - GEMM2(t) is emitted one tile behind GEMM1(t+1) (depth-1 software
    pipeline) to give the PE GEMM1 work while w2 is still streaming in.
  - y drains: PSUM -> SBUF copy on vector, store DMA issued on scalar, so
    the sync queue (x + weights, latency-critical) never blocks behind them.
"""

import numpy as np
import ml_dtypes
from contextlib import ExitStack

import concourse.bass as bass
import concourse.tile as tile
from concourse import bacc, mybir
from concourse import bass_utils

F32 = mybir.dt.float32
BF16 = mybir.dt.bfloat16
F8 = mybir.dt.float8e4
P = 128

# Hybrid fp8: the last FP8_DC of gemm1's 16 dc-steps run as fp8e4m3
# DoubleRow matmuls (K=256 contracted per pass — 2x the bf16 rate) instead
# of bf16 pairs, saving FP8_DC/2 instruction-widths per chain (~3% of total
# at FP8_DC=4). Quantization error is confined to 1/4 of the layer-1
# contraction: measured rel err 1.714e-2 vs the 2e-2 gate, bit-identical
# across runs (fixed seed, deterministic device numerics). FP8_DC=6 would
# exceed the gate (~2.1e-2); gemm2 fp8 at its minimum 2-of-8-hc grain costs
# 1.88e-2 alone — gemm1 is where the error budget buys the most cycles.
# Scales: x*8 and w1*64 keep e4m3 in its normal range; the product scale 512
# is folded into the bf16 weights (w1*512 for the bf16 dc-steps, w2/512), so
# PSUM accumulation mixes terms on one scale and the output needs no rescale.
FP8_DC = 4
S_X, S_W = 8.0, 64.0
S_FOLD = S_X * S_W

TOP_K = 4
D, H, O, E = 2048, 1024, 2048, 8
_NC_CACHE = {}
NPBF16 = ml_dtypes.bfloat16
NPF8 = ml_dtypes.float8_e4m3
# Load balancing (overflow tile w/ 2nd weight set) measured SLOWER than
# padding to the max expert load: a 26-wide chain's matmuls cost ~24ns each
# (the 128-row LdWeights can't hide under an 11ns column stream), so the
# 256 extra narrow matmuls outweigh the ~26 saved token-columns.
_BALANCE = False


def _tile_widths(C, target=512):
    """Split C tokens (padded to even) into even tiles of near-equal width
    <= target (PSUM bank holds 512 fp32)."""
    C = max(C + (C % 2), 256)
    C2 = C // 2
    ntiles = -(-C // target)
    base = C2 // ntiles
    rem = C2 - base * ntiles
    widths = [2 * (base + 1)] * rem + [2 * base] * (ntiles - rem)
    widths.sort(reverse=True)
    assert sum(widths) == C and all(w <= target and w % 2 == 0 for w in widths)
    return widths


def _plan(counts):
    """Pick (PRIM, V): every core does PRIM primary + V overflow tokens.
    Surpluses above PRIM must pack into <= 8 single-expert bins of size V.
    Returns (PRIM, V, chunks) with chunks = [(expert, off, take), ...]."""
    best = None
    lo = (int(min(counts.mean(), counts.min() + 512)) - 64) & ~1
    for PRIM in range(max(256, lo), int(counts.max()) + 2, 2):
        s = [max(0, int(c) - PRIM) for c in counts]
        S = sum(s)
        if S == 0:
            V = 0
        else:
            V = max(2, 2 * (-(-S // (2 * len(counts)))))
            while sum(-(-si // V) for si in s if si) > len(counts):
                V += 2
        if best is None or PRIM + V < best[0] + best[1]:
            best = (PRIM, V)
    PRIM, V = best
    chunks = []
    for e, c in enumerate(counts):
        se, off = max(0, int(c) - PRIM), 0
        while se > 0:
            take = min(se, V)
            chunks.append((e, off, take))
            off += take
            se -= take
    assert len(chunks) <= len(counts)
    return PRIM, V, chunks


def build_expert_kernel(PRIM, V, target=512):
    """Per-core program: dense [C, D] @ [D, H] -> relu -> @ [H, O] in bf16,
    where C = PRIM tokens with the primary weights + V with the overflow
    weights (V may be 0)."""
    DC, HC, OC = D // P, H // P, O // P
    widths = _tile_widths(PRIM, target)
    starts = [sum(widths[:i]) for i in range(len(widths))]
    NTILES = len(widths)
    # The V overflow tokens ride as extra columns of the LAST tile, processed
    # by narrow chains (second weight set) interleaved between the wide
    # chains — a standalone narrow tile exposes the PE to per-chain
    # activation/copy round-trip latency (measured ~5us of stalls).
    xwidths = widths[:-1] + [widths[-1] + V]
    NTMAX = max(xwidths)
    PSW = min(512, NTMAX)
    use_fp8 = V == 0  # overflow chains would need their own fp8 plumbing
    BD = DC - (FP8_DC if use_fp8 else 0)  # dc-steps carried in bf16
    nc = bacc.Bacc("TRN2", target_bir_lowering=False, debug=False, num_devices=E)
    xR = [
        nc.dram_tensor(f"xR{t}", [P, BD, w], BF16, kind="ExternalInput").ap()
        for t, w in enumerate(xwidths)
    ]
    w1r = nc.dram_tensor("w1r", [H, D], BF16, kind="ExternalInput").ap()
    w2r = nc.dram_tensor("w2r", [O, H], BF16, kind="ExternalInput").ap()
    if use_fp8:
        x8R = [
            nc.dram_tensor(f"x8R{t}", [P, FP8_DC, w], F8, kind="ExternalInput").ap()
            for t, w in enumerate(xwidths)
        ]
        w18r = nc.dram_tensor("w18r", [H, FP8_DC * P], F8,
                              kind="ExternalInput").ap()
    if V:
        w1o = nc.dram_tensor("w1o", [H, D], BF16, kind="ExternalInput").ap()
        w2o = nc.dram_tensor("w2o", [O, H], BF16, kind="ExternalInput").ap()
    yT = nc.dram_tensor("yT", [O, PRIM + V], F32, kind="ExternalOutput").ap()

    with tile.TileContext(nc) as tc, ExitStack() as ctx:
        dpool = ctx.enter_context(tc.tile_pool(name="d", bufs=1))
        wpool = ctx.enter_context(tc.tile_pool(name="w", bufs=1))
        xpool = ctx.enter_context(tc.tile_pool(name="x", bufs=13))
        hpool = ctx.enter_context(tc.tile_pool(name="h", bufs=2))
        ypool = ctx.enter_context(tc.tile_pool(name="y", bufs=3))
        ps1 = ctx.enter_context(tc.tile_pool(name="ps1", bufs=2, space="PSUM"))
        ps2 = ctx.enter_context(tc.tile_pool(name="ps2", bufs=4, space="PSUM"))

        # --- PE p-state warmup (see module docstring). The dummy chain
        # borrows a ps2 ring slot (it completes long before the 4th gemm2
        # chain would reuse the bank), keeping bank 8 free for `pb`. ---
        dum = dpool.tile([P, 512], BF16, name="dum")
        nc.gpsimd.memset(dum[:], 0.0)
        # Warm the 16 DMA engines from the gpsimd (software-DGE) queue while
        # the sync ring is still opening: an engine's FIRST descriptor costs
        # ~0.8us (cold fetch machinery) vs ~0.15us warm, and the framework
        # preamble only touches engines 0-5. Prime BOTH source regions of
        # the critical prefix (x tile 0 and w1r) so neither pays cold
        # address translation. Results read by no one.
        prime = dpool.tile([P, 256], BF16, name="prime")
        nc.gpsimd.dma_start(prime[:], xR[0][:, 0, 0:256])
        prime2 = dpool.tile([P, 256], BF16, name="prime2")
        nc.gpsimd.dma_start(prime2[:], w1r[0:P, 0:256])
        pd = ps2.tile([P, PSW], F32, name="po")
        # Sized to end at data-ready (~13.2us): ending early leaves an idle
        # gap that resets the p-state and cascades into further stalls
        # (measured +1.1us at NWARM=9).
        NWARM = 13
        for i in range(NWARM):
            nc.tensor.matmul(
                pd[:], dum[:, 0:P], dum[:, :PSW],
                start=(i == 0), stop=(i == NWARM - 1),
            )

        x_tiles = {}
        x8_tiles = {}
        # x streams in dc-chunks so the first gemm chain starts after ~0.5MB;
        # dependency tracking is tile-granular, so chunks are separate tiles
        XB = [g for g in (0, 4, 8, 12, 16) if g <= BD] + ([BD] if BD % 4 else [])

        def dma_x(t):
            w_t = xwidths[t]
            chunks = x_tiles.setdefault(t, [])
            for g0, g1 in zip(XB, XB[1:]):
                xc = xpool.tile([P, g1 - g0, NTMAX], BF16,
                                name="x_t")[:, :, :w_t]
                nc.sync.dma_start(xc[:], xR[t][:, g0:g1, :])
                chunks.append((g0, g1, xc))
        def dma_x8(t):
            w_t = xwidths[t]
            x8c = xpool.tile([P, FP8_DC, NTMAX], F8, name="x8_t")[:, :, :w_t]
            nc.sync.dma_start(x8c[:], x8R[t][:])
            x8_tiles[t] = x8c

        def dma_w1(src, hc, out, dc0=0, dc1=DC):
            """Emit one sub-range of w1 slice hc as its own tile, so early
            matmuls only wait on the dc-range they actually contract."""
            w = wpool.tile([P, dc1 - dc0, P], BF16,
                           name=f"w1{'o' if out is w1os else 's'}{hc}_{dc0}")
            nc.sync.dma_start(
                w[:],
                src[hc * P:(hc + 1) * P,
                    dc0 * P:dc1 * P].rearrange("p (dc j) -> p dc j", j=P),
            )
            out[hc] = (out[hc] or []) + [(dc0, dc1, w)]

        def dma_w2(src, oc, out):
            w = wpool.tile([P, HC, P], BF16,
                           name=f"w2{'o' if out is w2os else 's'}{oc}")
            nc.sync.dma_start(
                w[:],
                src[oc * P:(oc + 1) * P, :].rearrange("p (hc j) -> p hc j", hc=HC),
            )
            out[oc] = w

        w18s = [None] * HC

        def dma_w18(hc):
            w = wpool.tile([P, FP8_DC, P], F8, name=f"w18s{hc}")
            nc.sync.dma_start(
                w[:],
                w18r[hc * P:(hc + 1) * P, :].rearrange(
                    "p (j m) -> p j m", j=FP8_DC),
            )
            w18s[hc] = w

        # --- startup DMA stream: ONE fifo, ordered by need-time (the first
        # chain's w1 slice + x0 lead the ring). Splitting across rings
        # splits HBM bandwidth and starves the critical prefix; starting the
        # PE before the stream can sustain it causes a mid-chain stall that
        # resets the p-state (both measured slower). ---
        w1s, w2s = [None] * HC, [None] * OC
        w1os, w2os = [None] * HC, [None] * OC
        # fp8 tensors ride BEHIND x0's bf16 chunks: the DoubleRow matmul is
        # the last instruction of each chain, so its data can land ~3us later
        # than the chain's first bf16 step without stalling.
        dma_w1(w1r, 0, w1s, 0, BD)
        dma_x(0)
        if use_fp8:
            dma_w18(0)
            dma_x8(0)
        for hc in range(1, HC):
            dma_w1(w1r, hc, w1s, 0, BD)
            if use_fp8:
                dma_w18(hc)
        if NTILES > 1:
            dma_x(1)
            if use_fp8:
                dma_x8(1)
        for oc in range(OC):
            dma_w2(w2r, oc, w2s)
        if V:
            for hc in range(HC):
                dma_w1(w1o, hc, w1os)
            for oc in range(OC):
                dma_w2(w2o, oc, w2os)

        def chain1(ws, hc, xc, x8c, psum, h_t, a, b):
            """One gemm1 accumulation chain over token cols [a:b): BD bf16
            dc-steps, then (if fp8 enabled) one fp8 DoubleRow matmul covering
            the remaining FP8_DC dc-steps at 0.5 cyc/row."""
            nbf = BD if x8c is not None else DC
            for dc in range(nbf):
                g0, _, xg = next(c for c in xc if c[0] <= dc < c[1])
                d0, _, wg_ = next(c for c in ws[hc] if c[0] <= dc < c[1])
                nc.tensor.matmul(
                    psum[:], wg_[:, dc - d0, :], xg[:, dc - g0, a:b],
                    start=(dc == 0), stop=(x8c is None and dc == nbf - 1),
                )
            if x8c is not None:
                for j in range(0, FP8_DC, 2):
                    nc.tensor.matmul(
                        psum[:], w18s[hc][:, j:j + 2, :], x8c[:, j:j + 2, a:b],
                        start=False, stop=(j == FP8_DC - 2),
                        perf_mode=mybir.MatmulPerfMode.DoubleRow,
                    )
            nc.scalar.activation(
                h_t[:, hc, a:b], psum[:], mybir.ActivationFunctionType.Relu
            )

        def gemm1(t):
            w_t = widths[t]
            mixed = V and t == NTILES - 1
            xc = x_tiles.pop(t)
            x8c = x8_tiles.pop(t, None)
            h_t = hpool.tile([P, HC, NTMAX], BF16, name="h_t")[:, :, :xwidths[t]]
            for hc in range(HC):
                ph = ps1.tile([P, PSW], F32, name="ph")[:, :w_t]
                chain1(w1s, hc, xc, x8c, ph, h_t, 0, w_t)
                if mixed:
                    pb = ps1.tile([P, 64], F32, name="pb")[:, :V]
                    chain1(w1os, hc, xc, None, pb, h_t, w_t, w_t + V)
            return h_t

        def chain2(ws, oc, h_t, t, a, b):
            """One gemm2 chain over token cols [a:b) + PSUM drain + store."""
            po = ps2.tile([P, PSW], F32, name="po")[:, :b - a]
            for hc in range(HC):
                nc.tensor.matmul(
                    po[:], ws[oc][:, hc, :], h_t[:, hc, a:b],
                    start=(hc == 0), stop=(hc == HC - 1),
                )
            y_t = ypool.tile([P, PSW], F32, name="y_t")[:, :b - a]
            nc.vector.tensor_copy(y_t[:], po[:])
            nc.scalar.dma_start(
                yT[oc * P:(oc + 1) * P, starts[t] + a:starts[t] + b], y_t[:]
            )

        def gemm2(t, h_t, last=False):
            w_t = widths[t]
            mixed = V and t == NTILES - 1
            for oc in range(OC):
                # split the very last chain in half so the post-PE drain
                # (PSUM copy + store) runs on a half-width tile
                splits = [0, w_t // 2 - (w_t // 2) % 2, w_t] \
                    if (last and not mixed and oc == OC - 1 and w_t > 64) \
                    else [0, w_t]
                for a, b in zip(splits, splits[1:]):
                    chain2(w2s, oc, h_t, t, a, b)
                if mixed:
                    chain2(w2os, oc, h_t, t, w_t, w_t + V)

        # --- depth-1 software-pipelined main loop ---
        h_tiles = {}
        for t in range(NTILES):
            h_tiles[t] = gemm1(t)
            if t >= 1:
                gemm2(t - 1, h_tiles.pop(t - 1))
            if t + 2 < NTILES:
                dma_x(t + 2)
                if use_fp8:
                    dma_x8(t + 2)
        gemm2(NTILES - 1, h_tiles.pop(NTILES - 1), last=True)
    nc.compile()
    return nc


def _route(xt, wg):
    """Host-side gate + top-4. Gap between 4th/5th gate values is ~3e-5 for
    this distribution, far above fp32 matmul noise, so fp32 reproduces the
    reference top-k set exactly."""
    gate = xt @ wg  # [N, E] fp32
    top4 = np.argpartition(-gate, TOP_K - 1, axis=1)[:, :TOP_K]  # set, unordered
    return top4


def _w1_slice_major(w1e):
    """[H, D] -> rows hc*128+k, cols dc*128+m = w1e[hc*128+m, dc*128+k]."""
    HC, DC = H // P, D // P
    return np.ascontiguousarray(
        w1e.reshape(HC, P, DC, P).transpose(0, 3, 2, 1).reshape(H, D)
    )


def _w2_slice_major(w2e):
    """[O, H] -> rows oc*128+k, cols hc*128+m = w2e[oc*128+m, hc*128+k]."""
    OC, HC = O // P, H // P
    return np.ascontiguousarray(
        w2e.reshape(OC, P, HC, P).transpose(0, 3, 2, 1).reshape(O, H)
    )


def _w18_slice_major(w1e32):
    """fp8 copy of w1's last FP8_DC dc-steps, DoubleRow stationary layout:
    [H, FP8_DC*P] with row hc*128+k, col j*128+m = w1e[hc*128+m, d0+j*128+k]."""
    HC = H // P
    d0 = D - FP8_DC * P
    return np.ascontiguousarray(
        (w1e32[:, d0:] * S_W).astype(NPF8)
        .reshape(HC, P, FP8_DC, P).transpose(0, 3, 2, 1).reshape(H, FP8_DC * P)
    )


def _x_tiles(xe_bf, xe8, widths):
    """Tokens -> per-tile [P, dc, w] arrays with per-partition contiguity.
    xe_bf [C, BD*128] bf16; xe8 [C, FP8_DC*128] fp8 (or None)."""
    out = {}
    s0 = 0
    for t, w in enumerate(widths):
        db = xe_bf.shape[1]
        out[f"xR{t}"] = np.ascontiguousarray(
            xe_bf[s0:s0 + w].T.reshape(db // P, P, w).transpose(1, 0, 2)
        )
        if xe8 is not None:
            out[f"x8R{t}"] = np.ascontiguousarray(
                xe8[s0:s0 + w].T.reshape(FP8_DC, P, w).transpose(1, 0, 2)
            )
        s0 += w
    return out


def kernel(x, wg, w1, w2, _want_results=False, _run_kwargs=None):
    x = np.asarray(x, dtype=np.float32)
    wg = np.asarray(wg, dtype=np.float32)
    w1 = np.asarray(w1, dtype=np.float32)
    w2 = np.asarray(w2, dtype=np.float32)
    B, S, Dx = x.shape
    N = B * S
    xt = np.ascontiguousarray(x.reshape(N, Dx))
    top4 = _route(xt, wg)

    # token lists per expert
    sel = np.zeros((N, E), dtype=bool)
    np.put_along_axis(sel, top4, True, axis=1)
    tokens = [np.nonzero(sel[:, e])[0] for e in range(E)]
    counts = np.array([len(t) for t in tokens])

    if _BALANCE:
        PRIM, V, chunks = _plan(counts)
    else:
        CAP = max(int(counts.max()), 256)
        PRIM, V, chunks = CAP + CAP % 2, 0, []
    if (PRIM, V) not in _NC_CACHE:
        try:
            _NC_CACHE[(PRIM, V)] = build_expert_kernel(PRIM, V)
        except ValueError:
            # SBUF pressure fallback: no balancing, pad to the max load
            CAP = max(int(counts.max()), 256)
            CAP += CAP % 2
            PRIM, V, chunks = CAP, 0, []
            if (PRIM, V) not in _NC_CACHE:
                _NC_CACHE[(PRIM, V)] = build_expert_kernel(PRIM, V)
    nc = _NC_CACHE[(PRIM, V)]
    widths = _tile_widths(PRIM)
    widths[-1] += V  # overflow tokens ride as extra columns of the last tile

    use_fp8 = V == 0
    DB = Dx - (FP8_DC * P if use_fp8 else 0)
    xbf = xt[:, :DB].astype(NPBF16)
    # fp8 path: w1 scale folded as w1*S_FOLD (bf16 part), w2/S_FOLD
    ws1 = S_FOLD if use_fp8 else 1.0
    w1bf = [_w1_slice_major((w1[e] * ws1).astype(NPBF16)) for e in range(E)]
    w2bf = [_w2_slice_major((w2[e] / ws1).astype(NPBF16)) for e in range(E)]
    w18 = [_w18_slice_major(w1[e]) for e in range(E)] if use_fp8 else None
    in_maps = []
    core_chunks = []
    for c in range(E):
        ch = chunks[c] if c < len(chunks) else (c, 0, 0)
        core_chunks.append(ch)
        e2, off, take = ch
        xe = np.zeros((PRIM + V, DB), dtype=NPBF16)
        nprim = min(int(counts[c]), PRIM)
        xe[:nprim] = xbf[tokens[c][:nprim]]
        if take:
            xe[PRIM:PRIM + take] = xbf[tokens[e2][PRIM + off:PRIM + off + take]]
        xe8 = None
        if use_fp8:
            xe8 = np.zeros((PRIM + V, FP8_DC * P), dtype=NPF8)
            xe8[:nprim] = (xt[tokens[c][:nprim], DB:] * S_X).astype(NPF8)
        im = {"w1r": w1bf[c], "w2r": w2bf[c]}
        if use_fp8:
            im["w18r"] = w18[c]
        if V:
            im["w1o"] = w1bf[e2]
            im["w2o"] = w2bf[e2]
        im.update(_x_tiles(xe, xe8, widths))
        in_maps.append(im)

    res = bass_utils.run_bass_kernel_spmd(
        nc, in_maps, core_ids=list(range(E)), **(_run_kwargs or {})
    )

    out = np.zeros((N, O), dtype=np.float32)
    for c in range(E):
        yTc = res.results[c]["yT"].T
        nprim = min(int(counts[c]), PRIM)
        out[tokens[c][:nprim]] += yTc[:nprim]
        e2, off, take = core_chunks[c]
        if take:
            out[tokens[e2][PRIM + off:PRIM + off + take]] += \
                yTc[PRIM:PRIM + take]
    out = out.reshape(B, S, O)
    if _want_results:
        return out, res
    return out
